# revision 4
# baseline (speedup 1.0000x reference)
"""Bathtub reconstructor Trainium2 kernel.

Reference does, per (b, y, x, t) cell with its 16 fine topo values z_k:
    solve mean(relu(h - z)) = d by 20-step bisection, output relu(h - z_k).

Key identity (water-filling): with z sorted ascending and P_j = z_1+...+z_j,
    sum_k relu(h - z_k) = max_j (j*h - P_j)   (convex, increasing)
so the root of sum = 16*d is exactly
    h* = min_{j=1..16} (16*d + P_j) / j = min_j (a_j * d + b_j),
with a_j = 16/j (constants) and b_j = P_j/j (per-cell constants).
This replaces the 20-iteration bisection with 16 fused multiply-adds and a
16-way min, then the output pass relu(h* - z_k).

Device layout (per core, n_y sharded 8 ways -> 8 y-rows/core):
  partitions = 128 cells (4 tiles cover the 512 (y,x) cells)
  free dim   = 512 combos (b-major: b*32 + t)
  stage1: hj[j] = a_j*d + b_j   (scalar ACT: Identity w/ scale+bias, and
                                 vector tensor_scalar mult+add, split)
  stage2: h = min over j        (vector tensor_reduce, j innermost via AP)
  stage3: out[k] = relu(h - z_k) (vector tensor_scalar add+max / ACT Relu)
All DMAs fully contiguous; host pre/post-permutes (cheap numpy).
"""

import numpy as np

import concourse.bass as bass
import concourse.tile as tile
from concourse import bacc, mybir
from concourse.bass_utils import run_bass_kernel_spmd

BS, NY, NX, NT, F = 16, 64, 64, 32, 4
FF = F * F                # 16 fine cells per coarse cell
NCORES = 8
YPC = NY // NCORES        # 8 coarse y rows per core
CELLS = YPC * NX          # 512 cells per core
NCT = CELLS // 128        # 4 cell-tiles of 128 partitions
COMBOS = BS * NT          # 512 (b, t) combos per cell

F32 = mybir.dt.float32

# stage1 j-indices computed on vector (rest on scalar ACT); stage3 k-indices
# computed on scalar (rest on vector). Balances engine busy time around the
# vector-only 16-way min reduce.
S1_VEC = 3    # stage1: first S1_VEC j's on vector, rest on scalar
S3_SCA = 6    # stage3: first S3_SCA k's on scalar, rest on vector

_CACHE = {}


def _build_nc():
    nc = bacc.Bacc(
        "TRN2", target_bir_lowering=False, debug=False, num_devices=NCORES
    )
    u_ext = nc.declare_dram_parameter("u", [CELLS, COMBOS], F32, isOutput=False)
    cf_ext = nc.declare_dram_parameter("coef", [CELLS, FF], F32, isOutput=False)
    nz_ext = nc.declare_dram_parameter("negz", [CELLS, FF], F32, isOutput=False)
    out_ext = nc.declare_dram_parameter(
        "out", [CELLS, FF * COMBOS], F32, isOutput=True
    )

    a = [float(FF) / j for j in range(1, FF + 1)]

    with tile.TileContext(nc) as tc:
        with (
            tc.tile_pool(name="dpool", bufs=3) as dpool,
            tc.tile_pool(name="cfpool", bufs=3) as cfpool,
            tc.tile_pool(name="nzpool", bufs=3) as nzpool,
            tc.tile_pool(name="hjpool", bufs=2) as hjpool,
            tc.tile_pool(name="hpool", bufs=2) as hpool,
            tc.tile_pool(name="opool", bufs=2) as opool,
        ):
            for ct in range(NCT):
                rows = slice(128 * ct, 128 * (ct + 1))

                d = dpool.tile([128, COMBOS], F32)
                nc.sync.dma_start(d[:], u_ext[rows, :])
                cf = cfpool.tile([128, FF], F32)
                nc.sync.dma_start(cf[:], cf_ext[rows, :])
                nz = nzpool.tile([128, FF], F32)
                nc.sync.dma_start(nz[:], nz_ext[rows, :])

                # stage1: hj[:, j*COMBOS:(j+1)*COMBOS] = a_j * d + b_j
                hj = hjpool.tile([128, FF * COMBOS], F32)
                for j in range(FF):
                    o = hj[:, j * COMBOS:(j + 1) * COMBOS]
                    if j < S1_VEC:
                        nc.vector.tensor_scalar(
                            o, d[:], a[j], cf[:, j:j + 1],
                            op0=mybir.AluOpType.mult, op1=mybir.AluOpType.add,
                        )
                    else:
                        nc.scalar.activation(
                            o, d[:], mybir.ActivationFunctionType.Identity,
                            bias=cf[:, j:j + 1], scale=a[j],
                        )

                # stage2: h = min_j hj  (j innermost via strided AP view)
                h = hpool.tile([128, COMBOS], F32)
                hj_v = hj[:].rearrange("p (j c) -> p c j", j=FF)
                nc.vector.tensor_reduce(
                    h[:], hj_v, axis=mybir.AxisListType.X, op=mybir.AluOpType.min
                )

                # stage3: out[k] = relu(h - z_k)
                oa = opool.tile([128, FF * COMBOS], F32)
                for k in range(FF):
                    o = oa[:, k * COMBOS:(k + 1) * COMBOS]
                    if k < S3_SCA:
                        nc.scalar.activation(
                            o, h[:], mybir.ActivationFunctionType.Relu,
                            bias=nz[:, k:k + 1], scale=1.0,
                        )
                    else:
                        nc.vector.tensor_scalar(
                            o, h[:], nz[:, k:k + 1], 0.0,
                            op0=mybir.AluOpType.add, op1=mybir.AluOpType.max,
                        )

                nc.sync.dma_start(out_ext[rows, :], oa[:])
    nc.finalize()
    return nc


def _prep_inputs(u_coarse, topo):
    """Host-side: per-cell sorted-prefix coefficients + per-core shards."""
    u = np.ascontiguousarray(np.asarray(u_coarse, dtype=np.float32))
    tp = np.asarray(topo, dtype=np.float32)
    # fold fine topo into per-coarse-cell patches [NY, NX, FF]
    z = tp.reshape(NY, F, NX, F).transpose(0, 2, 1, 3).reshape(NY, NX, FF)
    zs = np.sort(z.astype(np.float64), axis=-1)
    pref = np.cumsum(zs, axis=-1)
    jj = np.arange(1, FF + 1, dtype=np.float64)
    coef = (pref / jj).astype(np.float32)          # [NY, NX, FF]
    negz = (-z).astype(np.float32)                 # [NY, NX, FF]

    in_maps = []
    for c in range(NCORES):
        ys = slice(c * YPC, (c + 1) * YPC)
        u_core = np.ascontiguousarray(
            u[:, ys, :, :].transpose(1, 2, 0, 3)
        ).reshape(CELLS, COMBOS)
        cf_core = np.ascontiguousarray(coef[ys]).reshape(CELLS, FF)
        nz_core = np.ascontiguousarray(negz[ys]).reshape(CELLS, FF)
        in_maps.append({"u": u_core, "coef": cf_core, "negz": nz_core})
    return in_maps


def _unshard(results):
    out_all = np.stack([r["out"] for r in results])          # [8, 512, 8192]
    arr = out_all.reshape(NCORES, YPC, NX, F, F, BS, NT)      # c,yl,x,fy,fx,b,t
    arr = arr.transpose(5, 0, 1, 3, 2, 4, 6)                  # b,c,yl,fy,x,fx,t
    return np.ascontiguousarray(arr).reshape(BS, NY * F, NX * F, NT)


def kernel(u_coarse, topo):
    if "nc" not in _CACHE:
        _CACHE["nc"] = _build_nc()
    nc = _CACHE["nc"]
    in_maps = _prep_inputs(u_coarse, topo)
    res = run_bass_kernel_spmd(nc, in_maps, core_ids=list(range(NCORES)))
    return _unshard(res.results)


if __name__ == "__main__":
    import reference

    inputs = reference.setup_inputs()
    out = kernel(**{k: np.asarray(v) for k, v in inputs.items()})
    print("out", out.shape, out.dtype)
